# revision 30
# baseline (speedup 1.0000x reference)
"""GATv2 graph layer Bass kernel for TRN2 (SPMD across 8 NeuronCores).

v4 design: edges sorted by destination and sharded across cores by dst
range. One fp16 gather table in DRAM (xsrc = node_emb @ W_src, p-major
layout for cheap 2KB-elem stores). Per edge chunk (128 edges, one dst
window, one src-table half):
  - gather xs rows (the ONLY per-edge DMA)
  - oh[e,d] one-hot of dst built on DVE (tensor_scalar is_equal)
  - ohT via PE transpose + PSUM->SBUF copy
  - combT[col,e] built on the PE in PSUM: emb8@oh8T + xdw@ohT + xs^T
    (kills the xd gather and the xd DRAM table of the old design)
  - lrelu via ACT Prelu(alpha=0.2) -> combT fp16 SBUF
  - logits = combT.T @ att_blk; ex = exp(logits - 4) (softmax-shift safe)
  - wgt = xs * ex (broadcast mul); scatter-accumulate per dst window via
    one-hot matmul into PSUM [sum_ex | sum_ex*xs]
  - window flush: agg/sum, @W_out (gamma-scaled), +beta +residual (all on
    PE), LayerNorm with rstd = exp(-0.5*ln(var+eps)) so the whole slab
    loop shares one ACT function table.
Output returned fp16 -> cast to fp32 on host.
"""
import numpy as np
from contextlib import ExitStack
from dataclasses import dataclass

import concourse.bass as bass
import concourse.tile as tile
from concourse import bacc, mybir
from concourse.masks import make_identity

P = 128
HID = 128
H = 4
HD = 32
NET = 8
EPS_LN = 1e-5
MAXCALL = 4096     # max idxs per dma_gather call
DEAD = -5.0        # dst_rel for padding slots
SPLIT = 25088      # src table half split (both halves < 32768 rows)
EXP_BIAS = -4.0    # constant logit shift (softmax invariant)


@dataclass
class Geo:
    N: int
    n_cores: int
    slab_w: int = 3

    @property
    def npc(self):
        return self.N // self.n_cores

    @property
    def nw(self):
        return (self.npc + P - 1) // P

    @property
    def nslab(self):
        return (self.nw + self.slab_w - 1) // self.slab_w

    @property
    def n_pad(self):   # table rows, 392*128 for N=50000
        return ((self.N + 1023) // 1024) * 1024


def wrap_idx(idx, cols):
    n = idx.shape[0]
    assert n % 16 == 0
    w = np.zeros((P, cols), dtype=np.int16)
    if n:
        t16 = idx.reshape(n // 16, 16).T
        for g in range(8):
            w[g * 16:(g + 1) * 16, :n // 16] = t16
    return w


def host_prep(g: Geo, node_embeddings, edge_index, edge_type, task_embedding,
              W_src, b_src, W_dst, b_dst, edge_emb, att,
              W_out, b_out, norm_w, norm_b, W_film, b_film):
    """Pure index work + tiny constant folding; O(N*HID)/O(E*HID) float
    math runs on device."""
    src = np.asarray(edge_index[0], dtype=np.int64)
    dst = np.asarray(edge_index[1], dtype=np.int64)
    et = np.asarray(edge_type, dtype=np.int64)
    npc, nw = g.npc, g.nw

    order = np.argsort(dst, kind="stable")
    src, dst, et = src[order], dst[order], et[order]
    core_of = dst // npc

    buckets = {}
    for c in range(g.n_cores):
        m = core_of == c
        cs, cd, ce = src[m], dst[m] - c * npc, et[m]
        for w in range(nw):
            wm = (cd // P) == w
            ws_, wd, we = cs[wm], cd[wm] - w * P, ce[wm]
            lo = ws_ < SPLIT
            buckets[(c, w, 0)] = (ws_[lo], wd[lo], we[lo])
            buckets[(c, w, 1)] = (ws_[~lo] - SPLIT, wd[~lo], we[~lo])

    caps = np.zeros((nw, 2), dtype=np.int64)
    for w in range(nw):
        for h in range(2):
            mx = max(len(buckets[(c, w, h)][0]) for c in range(g.n_cores))
            caps[w, h] = (mx + P - 1) // P
        if caps[w, 0] + caps[w, 1] == 0:
            caps[w, 0] = 1   # ensure every window gets a scatter group

    # ---- schedule ---------------------------------------------------------
    sched_slabs = []
    total_chunks = 0
    for s in range(g.nslab):
        ws = list(range(s * g.slab_w, min((s + 1) * g.slab_w, nw)))
        chunks = []            # (win_local, half, slot)
        calls = {0: [], 1: []}
        slot = 0
        for h in (0, 1):
            run = 0
            run_start = slot
            for w in ws:
                for _ in range(caps[w, h]):
                    chunks.append((w - ws[0], h, slot))
                    slot += 1
                    run += P
                    if run == MAXCALL:
                        calls[h].append((run_start, run))
                        run, run_start = 0, slot
            if run:
                calls[h].append((run_start, run))
        sched_slabs.append(dict(windows=ws, chunks=chunks, calls=calls,
                                chunk0=total_chunks))
        total_chunks += len(chunks)

    lo_cols = max(16, sum(n for sl in sched_slabs
                          for (_, n) in sl["calls"][0]) // 16)
    hi_cols = max(16, sum(n for sl in sched_slabs
                          for (_, n) in sl["calls"][1]) // 16)

    # ---- shared constants -------------------------------------------------
    # nodeT in p-major permuted column order: column (i*1024 + s*128 + p)
    # holds node (p*(n_pad//128) + i*8 + s), so that table-build block i
    # stores contiguous 2KB runs per partition and table row r == node r.
    npart_rows = g.n_pad // P          # rows per partition in p-major table
    nodeT_pm = np.zeros((HID, g.n_pad), dtype=np.float16)
    ne32 = np.asarray(node_embeddings, np.float32)
    cols = np.arange(g.n_pad)
    i_blk, rem = cols // 1024, cols % 1024
    s_sub, p_sub = rem // P, rem % P
    node_of_col = p_sub * npart_rows + i_blk * 8 + s_sub
    valid = node_of_col < g.N
    nodeT_pm[:, cols[valid]] = ne32[node_of_col[valid]].T.astype(np.float16)

    emb_eff = (np.asarray(edge_emb, np.float64)
               + np.asarray(b_src, np.float64)[None, :]
               + np.asarray(b_dst, np.float64)[None, :]).astype(np.float16)
    att_blk = np.zeros((HID, H), dtype=np.float16)
    for h in range(H):
        att_blk[h * HD:(h + 1) * HD, h] = np.asarray(att, np.float32)[h]

    consts = dict(
        nodeT_pm=nodeT_pm,
        W_src=np.asarray(W_src, np.float32).astype(np.float16),
        W_dst=np.asarray(W_dst, np.float32).astype(np.float16),
        W_out=np.asarray(W_out, np.float32).astype(np.float16),
        W_film=np.asarray(W_film, np.float32).astype(np.float16),
        b_film=np.asarray(b_film, np.float32).reshape(1, 2 * HID),
        b_out=np.asarray(b_out, np.float32).reshape(1, HID),
        task=np.asarray(task_embedding, np.float32).reshape(HID, 1)
            .astype(np.float16),
        emb8=emb_eff,                      # [8, HID] f16
        att_blk=att_blk,
    )
    skip_norm = bool(np.all(np.asarray(norm_w) == 1.0)
                     and np.all(np.asarray(norm_b) == 0.0))
    if not skip_norm:
        consts["normw"] = np.asarray(norm_w, np.float32).reshape(1, HID)
        consts["normb"] = np.asarray(norm_b, np.float32).reshape(1, HID)

    # ---- per-core arrays --------------------------------------------------
    in_maps = []
    for c in range(g.n_cores):
        lo_l, hi_l = [], []
        dstr = np.full((P, total_chunks), DEAD, dtype=np.float32)
        oh8T = np.zeros((NET, total_chunks * P), dtype=np.float16)
        ci = 0
        for sl in sched_slabs:
            ws0 = sl["windows"][0]
            per_half = {0: [], 1: []}
            nth = {}
            for (wl, h, slot) in sl["chunks"]:
                w = ws0 + wl
                es, ed, ee = buckets[(c, w, h)]
                k = nth.get((wl, h), 0)
                nth[(wl, h)] = k + 1
                sl_src = np.zeros(P, dtype=np.int64)
                n = min(P, max(0, len(es) - k * P))
                if n > 0:
                    sl_src[:n] = es[k * P:k * P + n]
                    dstr[:n, ci] = ed[k * P:k * P + n]
                    oh8T[ee[k * P:k * P + n], ci * P + np.arange(n)] = 1.0
                per_half[h].append(sl_src)
                ci += 1
            lo_l.extend(per_half[0])
            hi_l.extend(per_half[1])
        lo_i = (np.concatenate(lo_l) if lo_l else np.zeros(0, np.int64))
        hi_i = (np.concatenate(hi_l) if hi_l else np.zeros(0, np.int64))
        assert lo_i.max(initial=0) < SPLIT <= 32767 + 1
        assert hi_i.max(initial=0) < 32768

        m = dict(consts)
        no = np.zeros((nw * P, HID), dtype=np.float16)
        no[:npc] = ne32[c * npc:(c + 1) * npc].astype(np.float16)
        m["node_own"] = no                      # [nw*P, HID] f16
        m["lo_idx"] = wrap_idx(lo_i.astype(np.int16), lo_cols)
        m["hi_idx"] = wrap_idx(hi_i.astype(np.int16), hi_cols)
        m["dstr"] = dstr
        m["oh8T"] = oh8T
        in_maps.append(m)

    sched = dict(slabs=sched_slabs, caps=caps, total_chunks=total_chunks,
                 lo_cols=lo_cols, hi_cols=hi_cols, skip_norm=skip_norm)
    return sched, in_maps


def build_program(g: Geo, sched):
    nc = bacc.Bacc("TRN2", target_bir_lowering=False, debug=False,
                   num_devices=g.n_cores, num_swdge_queues=4)
    f16, f32 = mybir.dt.float16, mybir.dt.float32
    AF = mybir.ActivationFunctionType
    OP = mybir.AluOpType
    npc, nw = g.npc, g.nw
    total_chunks = sched["total_chunks"]
    lo_cols, hi_cols = sched["lo_cols"], sched["hi_cols"]
    cmax = max(len(sl["chunks"]) for sl in sched["slabs"])

    def din(name, shape, dt):
        return nc.dram_tensor(name, shape, dt, kind="ExternalInput").ap()

    nodeT_pm = din("nodeT_pm", [HID, g.n_pad], f16)
    node_own = din("node_own", [nw * P, HID], f16)
    W_src = din("W_src", [HID, HID], f16)
    W_dst = din("W_dst", [HID, HID], f16)
    W_out = din("W_out", [HID, HID], f16)
    W_film = din("W_film", [HID, 2 * HID], f16)
    b_film = din("b_film", [1, 2 * HID], f32)
    b_out = din("b_out", [1, HID], f32)
    task = din("task", [HID, 1], f16)
    emb8_d = din("emb8", [NET, HID], f16)
    att_d = din("att_blk", [HID, H], f16)
    lo_idx = din("lo_idx", [P, lo_cols], mybir.dt.int16)
    hi_idx = din("hi_idx", [P, hi_cols], mybir.dt.int16)
    dstr_d = din("dstr", [P, total_chunks], f32)
    oh8T_d = din("oh8T", [NET, total_chunks * P], f16)
    out = nc.dram_tensor("out", [nw * P, HID], f16, kind="ExternalOutput").ap()

    xsrc_pm = nc.dram_tensor("xsrc_pm", [g.n_pad, HID], f16).ap()

    with tile.TileContext(nc, trace_sim=False) as tc, ExitStack() as ctx:
        cpool = ctx.enter_context(tc.tile_pool(name="consts", bufs=1))
        bpool = ctx.enter_context(tc.tile_pool(name="build", bufs=3))
        spool = ctx.enter_context(tc.tile_pool(name="xs", bufs=3))
        hpool = ctx.enter_context(tc.tile_pool(name="oh", bufs=4))
        h8pool = ctx.enter_context(tc.tile_pool(name="oh8p", bufs=2))
        wpool = ctx.enter_context(tc.tile_pool(name="work", bufs=2))
        fpool = ctx.enter_context(tc.tile_pool(name="flush", bufs=3))
        opool = ctx.enter_context(tc.tile_pool(name="ostage", bufs=2))
        psM = ctx.enter_context(tc.tile_pool(name="psM", bufs=1, space="PSUM"))
        psC = ctx.enter_context(tc.tile_pool(name="psC", bufs=2, space="PSUM"))
        psT = ctx.enter_context(tc.tile_pool(name="psT", bufs=2, space="PSUM"))
        psE = ctx.enter_context(tc.tile_pool(name="psE", bufs=1, space="PSUM"))
        psW = ctx.enter_context(tc.tile_pool(name="psW", bufs=2, space="PSUM"))

        # ---- constants ----------------------------------------------------
        ident = cpool.tile([P, P], f16)
        make_identity(nc, ident[:])
        iota16 = cpool.tile([P, P], mybir.dt.int16)
        nc.gpsimd.iota(iota16[:], pattern=[[1, P]], base=0, channel_multiplier=0)
        iota = cpool.tile([P, P], f16)
        nc.vector.tensor_copy(iota[:], iota16[:])
        ones1 = cpool.tile([1, P], f16)
        nc.vector.memset(ones1[:], 1.0)
        eps_col = cpool.tile([P, 1], f32)
        nc.vector.memset(eps_col[:], EPS_LN)
        ebias_col = cpool.tile([P, 1], f32)
        nc.vector.memset(ebias_col[:], EXP_BIAS)

        Ws = cpool.tile([HID, HID], f16)
        nc.sync.dma_start(Ws[:], W_src[:])
        Wd = cpool.tile([HID, HID], f16)
        nc.sync.dma_start(Wd[:], W_dst[:])
        Wo = cpool.tile([HID, HID], f16)
        nc.sync.dma_start(Wo[:], W_out[:])
        Wf = cpool.tile([HID, 2 * HID], f16)
        nc.sync.dma_start(Wf[:], W_film[:])
        emb8 = cpool.tile([NET, HID], f16)
        nc.sync.dma_start(emb8[:], emb8_d[:])
        att_sb = cpool.tile([HID, H], f16)
        nc.sync.dma_start(att_sb[:], att_d[:])
        task_sb = cpool.tile([HID, 1], f16)
        nc.sync.dma_start(task_sb[:], task[:])
        bfilm_sb = cpool.tile([1, 2 * HID], f32)
        nc.sync.dma_start(bfilm_sb[:], b_film[:])
        bout_sb = cpool.tile([1, HID], f32)
        nc.sync.dma_start(bout_sb[:], b_out[:])
        lo_sb = cpool.tile([P, lo_cols], mybir.dt.int16)
        nc.sync.dma_start(lo_sb[:], lo_idx[:])
        hi_sb = cpool.tile([P, hi_cols], mybir.dt.int16)
        nc.sync.dma_start(hi_sb[:], hi_idx[:])
        dstr_sb = cpool.tile([P, total_chunks], f32)
        nc.sync.dma_start(dstr_sb[:], dstr_d[:])

        # node_own as [P, nw, HID] f16: partition p, window w -> node w*P+p
        node_own_sb = cpool.tile([P, nw, HID], f16)
        nc.sync.dma_start(node_own_sb[:],
                          node_own[:].rearrange("(w p) h -> p w h", p=P))

        # ---- FiLM (tanh table, done first) --------------------------------
        ps_f = psM.tile([1, 2 * HID], f32, space="PSUM", tag="pm")
        nc.tensor.matmul(out=ps_f[:], lhsT=task_sb[:], rhs=Wf[:],
                         start=True, stop=True)
        film = cpool.tile([1, 2 * HID], f32)
        nc.vector.tensor_add(film[:], ps_f[:], bfilm_sb[:])
        gam_t = cpool.tile([1, HID], f32)
        nc.scalar.activation(gam_t[:], film[:, :HID], AF.Tanh)
        gam16 = cpool.tile([1, HID], f16)
        nc.vector.tensor_scalar(gam16[:], gam_t[:], 0.5, 1.0, OP.mult, OP.add)
        tmpb = cpool.tile([1, HID], f32)
        nc.vector.tensor_mul(tmpb[:], bout_sb[:], gam16[:])
        beta16 = cpool.tile([1, HID], f16)
        nc.vector.tensor_add(beta16[:], tmpb[:], film[:, HID:])
        ps_g = psM.tile([P, HID], f32, space="PSUM", tag="pm")
        nc.tensor.matmul(out=ps_g[:], lhsT=ones1[:], rhs=gam16[:],
                         start=True, stop=True)
        gam_rep = cpool.tile([P, HID], f16)
        nc.vector.tensor_copy(gam_rep[:], ps_g[:])
        Wosc = cpool.tile([HID, HID], f16)
        nc.vector.tensor_mul(Wosc[:], Wo[:], gam_rep[:])

        if not sched["skip_norm"]:
            nw_dr = din("normw", [1, HID], f32)
            nb_dr = din("normb", [1, HID], f32)
            nw_sb = cpool.tile([1, HID], f32)
            nc.sync.dma_start(nw_sb[:], nw_dr[:])
            nb_sb = cpool.tile([1, HID], f32)
            nc.sync.dma_start(nb_sb[:], nb_dr[:])
            ps_w = psM.tile([P, 2 * HID], f32, space="PSUM", tag="pm")
            nc.tensor.matmul(out=ps_w[:, 0:HID], lhsT=ones1[:], rhs=nw_sb[:],
                             start=True, stop=True, skip_group_check=True)
            nc.tensor.matmul(out=ps_w[:, HID:], lhsT=ones1[:], rhs=nb_sb[:],
                             start=True, stop=True, skip_group_check=True)
            wb_rep = cpool.tile([P, 2 * HID], f32)
            nc.vector.tensor_copy(wb_rep[:], ps_w[:])

        # ---- xsrc table build (p-major, 2KB elems) ------------------------
        def _copy_act(o, i):
            nc.scalar.activation(o, i, AF.Copy)

        copy_rr = [lambda o, i: nc.vector.tensor_copy(o, i), _copy_act]
        for i in range(g.n_pad // 1024):
            nt = bpool.tile([HID, 1024], f16, tag="nt")
            nc.sync.dma_start(nt[:], nodeT_pm[:, i * 1024:(i + 1) * 1024])
            xt = bpool.tile([P, 8, HID], f16, tag="xt")
            for r in range(2):
                ps = psC.tile([P, 512], f32, space="PSUM", tag="pc")
                for j in range(4):
                    nc.tensor.matmul(out=ps[:, j * HID:(j + 1) * HID],
                                     lhsT=nt[:, (r * 4 + j) * P:(r * 4 + j + 1) * P],
                                     rhs=Ws[:], start=True, stop=True,
                                     skip_group_check=True)
                copy_rr[(2 * i + r) % 2](
                    xt[:, r * 4:(r + 1) * 4, :],
                    ps[:].rearrange("p (s h) -> p s h", s=4))
            nc.sync.dma_start(
                xsrc_pm[:].rearrange("(p j) h -> p j h", p=P)
                [:, i * 8:(i + 1) * 8, :], xt[:])
        xsrc_flat = xsrc_pm

        # ---- xdw_all: x_dst per window, kept in SBUF ----------------------
        xdw_all = cpool.tile([P, nw, HID], f16)
        for w in range(nw):
            pst = psT.tile([P, P], f16, space="PSUM", tag="pt")
            nc.tensor.transpose(out=pst[:], in_=node_own_sb[:, w, :],
                                identity=ident[:])
            noT = bpool.tile([HID, P], f16, tag="noT")
            copy_rr[w % 2](noT[:], pst[:])
            ps = psM.tile([P, HID], f32, space="PSUM", tag="pm")
            nc.tensor.matmul(out=ps[:], lhsT=noT[:], rhs=Wd[:],
                             start=True, stop=True)
            copy_rr[(w + 1) % 2](xdw_all[:, w, :], ps[:])

        # ---- edge slabs ----------------------------------------------------
        off16 = {0: 0, 1: 0}
        qn = [0]

        def nextq():
            qn[0] = (qn[0] + 1) % 4
            return qn[0]

        sl0 = sched["slabs"][0]
        C00 = len(sl0["chunks"])
        t8 = h8pool.tile([NET, C00 * P], f16, tag="oh8", name="oh8_0")
        nc.sync.dma_start(t8[:], oh8T_d[:, 0:C00 * P])
        oh8_tiles = [t8]
        pending_flush = [None]

        for s, sl in enumerate(sched["slabs"]):
            ws = sl["windows"]
            nwin = len(ws)
            chunks = sl["chunks"]
            C = len(chunks)
            c0 = sl["chunk0"]
            ngrp = (C + 3) // 4

            # per-slab type one-hots [8, C*128], prefetched one slab ahead
            oh8_sb = oh8_tiles.pop(0)
            if s + 1 < len(sched["slabs"]):
                nsl = sched["slabs"][s + 1]
                nC = len(nsl["chunks"])
                nt8 = h8pool.tile([NET, nC * P], f16, tag="oh8",
                                  name=f"oh8_{s + 1}")
                nc.sync.dma_start(
                    nt8[:], oh8T_d[:, nsl["chunk0"] * P:
                                   (nsl["chunk0"] + nC) * P])
                oh8_tiles.append(nt8)

            # gathers (xs rows; only per-edge DMA)
            xs_t = spool.tile([P, C, HID], f16, tag="xs")
            for h in (0, 1):
                base = SPLIT if h == 1 else 0
                idx_sb = lo_sb if h == 0 else hi_sb
                for (slot_off, n) in sl["calls"][h]:
                    if n == 0:
                        continue
                    nc.gpsimd.dma_gather(
                        out_ap=xs_t[:, slot_off:slot_off + n // P, :],
                        in_ap=xsrc_flat[base:g.n_pad, :],
                        idxs_ap=idx_sb[:, off16[h]:off16[h] + n // 16],
                        num_idxs=n, num_idxs_reg=n, elem_size=HID,
                        single_packet=(n <= 1024), queue_num=nextq(),
                    )
                    off16[h] += n // 16

            # one-hot builds (DVE 4x tensor_scalar)
            oh_all = hpool.tile([P, C, P], f16, tag="oh")
            for c in range(C):
                eng = nc.vector if c % 3 != 2 else nc.gpsimd
                eng.tensor_scalar(oh_all[:, c, :], iota[:],
                                  dstr_sb[:, c0 + c:c0 + c + 1], None,
                                  OP.is_equal)

            # ohT via PE transpose + grouped PSUM->SBUF copy
            ohT_all = hpool.tile([P, C, P], f16, tag="ohT")
            for gi in range(ngrp):
                gn = min(4, C - gi * 4)
                pst = psT.tile([P, 4, P], f16, space="PSUM", tag="pt")
                for k in range(gn):
                    nc.tensor.transpose(out=pst[:, k, :],
                                        in_=oh_all[:, gi * 4 + k, :],
                                        identity=ident[:])
                if gi % 2 == 0:
                    nc.vector.tensor_copy(ohT_all[:, gi * 4:gi * 4 + gn, :],
                                          pst[:, :gn, :])
                else:
                    nc.scalar.activation(ohT_all[:, gi * 4:gi * 4 + gn, :],
                                         pst[:, :gn, :], AF.Copy)

            # combT groups: emb8@oh8T + xdw@ohT + xs^T, then lrelu
            combT = wpool.tile([P, C, HID], f16, tag="combT")
            for gi in range(ngrp):
                gn = min(4, C - gi * 4)
                pc = psC.tile([P, 4, HID], f32, space="PSUM", tag="pc")
                nc.tensor.matmul(
                    out=pc[:, :gn, :].rearrange("p a b -> p (a b)"),
                    lhsT=emb8[:],
                    rhs=oh8_sb[:, (gi * 4) * P:(gi * 4 + gn) * P],
                    start=True, stop=False, skip_group_check=True)
                k = 0
                while k < gn:
                    c = gi * 4 + k
                    wl = chunks[c][0]
                    k2 = k + 1
                    while k2 < gn and chunks[gi * 4 + k2][0] == wl:
                        k2 += 1
                    nc.tensor.matmul(
                        out=pc[:, k:k2, :].rearrange("p a b -> p (a b)"),
                        lhsT=xdw_all[:, ws[0] + wl, :],
                        rhs=ohT_all[:, c:gi * 4 + k2, :]
                        .rearrange("p a b -> p (a b)"),
                        start=False, stop=False, skip_group_check=True)
                    k = k2
                for k in range(gn):
                    c = gi * 4 + k
                    nc.tensor.matmul(out=pc[:, k, :],
                                     lhsT=xs_t[:, c, :], rhs=ident[:],
                                     start=False, stop=True,
                                     skip_group_check=True)
                nc.scalar.activation(combT[:, gi * 4:gi * 4 + gn, :],
                                     pc[:, :gn, :], AF.Prelu, alpha=0.2)

            # logits + exp
            ex_ps = psE.tile([P, C, H], f32, space="PSUM", tag="pe")
            for c in range(C):
                nc.tensor.matmul(out=ex_ps[:, c, :], lhsT=combT[:, c, :],
                                 rhs=att_sb[:], start=True, stop=True,
                                 skip_group_check=True)
            rhs_t = wpool.tile([P, C, 4 + HID], f16, tag="rhs")
            nc.scalar.activation(rhs_t[:, :, 0:4], ex_ps[:], AF.Exp,
                                 bias=ebias_col[:])

            # weighted messages
            nc.vector.tensor_mul(
                rhs_t[:, :, 4:].rearrange("p c (h d) -> p c h d", h=H),
                xs_t[:].rearrange("p c (h d) -> p c h d", h=H),
                rhs_t[:, :, 0:4].unsqueeze(3).broadcast_to([P, C, H, HD]))

            # scatter-accumulate per window
            win_t = [psW.tile([P, 2, 4 + HID], f32, space="PSUM", tag="win",
                              name=f"win{s}_{i}")
                     for i in range((nwin + 1) // 2)]
            win_ps = [win_t[i // 2][:, i % 2, :] for i in range(nwin)]
            # window-by-window emission: a PSUM bank holds two windows and
            # tolerates only ONE open accumulation group at a time.
            n_per_win = [0] * nwin
            for (wl, h, slot) in chunks:
                n_per_win[wl] += 1
            seen = [0] * nwin
            flush_jobs = []
            by_win = sorted(range(C), key=lambda c: (chunks[c][0], c))
            for c in by_win:
                (wl, h, slot) = chunks[c]
                first = seen[wl] == 0
                seen[wl] += 1
                last = seen[wl] == n_per_win[wl]
                nc.tensor.matmul(out=win_ps[wl], lhsT=oh_all[:, slot, :],
                                 rhs=rhs_t[:, slot, :], start=first, stop=last,
                                 skip_group_check=True)
                if last:
                    flush_jobs.append(wl)

            def make_flush(s, ws, nwin, win_ps, flush_jobs):
              def flush():
                self_flush(s, ws, nwin, win_ps, flush_jobs)
              return flush
            pending_flush.append(make_flush(s, ws, nwin, win_ps, flush_jobs))

        # final flush
        for fj in pending_flush:
            if fj is not None:
                fj()
        if False:
            # ---- flush windows (moved into self_flush) --------------------
            ysl = opool.tile([P, nwin, HID], f16, tag="ysl", name=f"ysl{s}")
            stat = fpool.tile([P, 3, nwin], f32, tag="stat", name=f"stat{s}")
            sum_y, sum_sq, rstd_sl = stat[:, 0, :], stat[:, 1, :], stat[:, 2, :]
            for wl in flush_jobs:
                w = ws[0] + wl
                pw = win_ps[wl]
                sums = fpool.tile([P, 4], f32, tag="sums")
                nc.vector.tensor_scalar(sums[:], pw[:, 0:4], 1e-12, None,
                                        OP.max)
                rec = fpool.tile([P, 4], f32, tag="rec")
                nc.vector.reciprocal(rec[:], sums[:])
                aggn = fpool.tile([P, HID], f16, tag="aggn")
                nc.vector.tensor_mul(
                    aggn[:].rearrange("p (h d) -> p h d", h=H),
                    pw[:, 4:].rearrange("p (h d) -> p h d", h=H),
                    rec[:].unsqueeze(2).broadcast_to([P, H, HD]))
                pst = psT.tile([P, P], f16, space="PSUM", tag="pt")
                nc.tensor.transpose(out=pst[:], in_=aggn[:], identity=ident[:])
                aggT = fpool.tile([HID, P], f16, tag="aggT")
                nc.vector.tensor_copy(aggT[:], pst[:])
                po = psM.tile([P, HID], f32, space="PSUM", tag="pm")
                nc.tensor.matmul(out=po[:], lhsT=aggT[:], rhs=Wosc[:],
                                 start=True, stop=False, skip_group_check=True)
                nc.tensor.matmul(out=po[:], lhsT=ones1[:], rhs=beta16[:],
                                 start=False, stop=False, skip_group_check=True)
                nc.tensor.matmul(out=po[:], lhsT=ident[:],
                                 rhs=node_own_sb[:, w, :],
                                 start=False, stop=True, skip_group_check=True)
                nc.vector.tensor_scalar(ysl[:, wl, :], po[:], 1.0, None,
                                        OP.mult,
                                        accum_out=sum_y[:, wl:wl + 1])
                sq = fpool.tile([P, HID], f16, tag="sq")
                nc.scalar.activation(sq[:], ysl[:, wl, :], AF.Square,
                                     accum_out=sum_sq[:, wl:wl + 1])
            # rstd = exp(-0.5*ln(var+eps)); var = sumsq/H - mu^2
            mu_sl = fpool.tile([P, nwin], f32, tag="mu", name=f"mu{s}")
            nc.vector.tensor_scalar(mu_sl[:], sum_y[:], 1.0 / HID, None,
                                    OP.mult)
            var_sl = fpool.tile([P, nwin], f32, tag="var")
            nc.vector.tensor_mul(var_sl[:], mu_sl[:], mu_sl[:])
            nc.vector.tensor_scalar(var_sl[:], var_sl[:], -1.0, None, OP.mult)
            nc.vector.scalar_tensor_tensor(
                out=var_sl[:], in0=sum_sq[:], scalar=1.0 / HID, in1=var_sl[:],
                op0=OP.mult, op1=OP.add)
            # rstd = rsqrt(var+eps) via bit-trick seed + 2 Newton iters on
            # DVE (keeps ACT on a single function table: no Ln/Sqrt).
            nc.vector.tensor_scalar(var_sl[:], var_sl[:], EPS_LN, None, OP.add)
            sd_t = fpool.tile([P, nwin], f32, tag="lnv")
            nc.vector.tensor_scalar(sd_t[:].bitcast(mybir.dt.int32),
                                    var_sl[:].bitcast(mybir.dt.int32),
                                    1, None, OP.logical_shift_right)
            nc.vector.tensor_scalar(sd_t[:].bitcast(mybir.dt.int32),
                                    sd_t[:].bitcast(mybir.dt.int32),
                                    -1, 0x5F3759DF, OP.mult, OP.add)
            half_v = fpool.tile([P, nwin], f32, tag="halfv")
            nc.vector.tensor_scalar(half_v[:], var_sl[:], -0.5, None, OP.mult)
            for _ in range(2):
                y2 = fpool.tile([P, nwin], f32, tag="y2")
                nc.vector.tensor_mul(y2[:], sd_t[:], sd_t[:])
                nc.vector.tensor_mul(y2[:], y2[:], half_v[:])
                nc.vector.tensor_scalar(y2[:], y2[:], 1.5, None, OP.add)
                nc.vector.tensor_mul(sd_t[:], sd_t[:], y2[:])
            nc.vector.tensor_copy(rstd_sl, sd_t[:])
            for wl in range(nwin):
                nc.vector.tensor_scalar(ysl[:, wl, :], ysl[:, wl, :],
                                        mu_sl[:, wl:wl + 1],
                                        rstd_sl[:, wl:wl + 1],
                                        OP.subtract, OP.mult)
            if not sched["skip_norm"]:
                for wl in range(nwin):
                    nc.vector.tensor_mul(ysl[:, wl, :], ysl[:, wl, :],
                                         wb_rep[:, 0:HID])
                    nc.vector.tensor_add(ysl[:, wl, :], ysl[:, wl, :],
                                         wb_rep[:, HID:])
            nc.sync.dma_start(
                out[ws[0] * P:(ws[0] + nwin) * P, :]
                .rearrange("(w p) h -> p w h", p=P), ysl[:])

    nc.compile()
    return nc


_CACHE = {}


def kernel(**inputs):
    N = int(np.asarray(inputs["node_embeddings"]).shape[0])
    n_cores = 8
    g = Geo(N=N, n_cores=n_cores)

    sched, in_maps = host_prep(g, **{k: np.asarray(v) for k, v in inputs.items()})

    key = (N, sched["total_chunks"], tuple(int(x) for x in sched["caps"].ravel()),
           sched["skip_norm"])
    if key not in _CACHE:
        _CACHE[key] = build_program(g, sched)
    nc = _CACHE[key]

    from concourse.bass_utils import run_bass_kernel_spmd
    res = run_bass_kernel_spmd(nc, in_maps, core_ids=list(range(n_cores)))
    out = np.concatenate([np.asarray(res.results[c]["out"][:g.npc],
                                     np.float32)
                          for c in range(n_cores)], axis=0)
    return out


# revision 32
# speedup vs baseline: 1.0254x; 1.0254x over previous
"""GATv2 graph layer Bass kernel for TRN2 (SPMD across 8 NeuronCores).

v4 design: edges sorted by destination and sharded across cores by dst
range. One fp16 gather table in DRAM (xsrc = node_emb @ W_src, p-major
layout for cheap 2KB-elem stores). Per edge chunk (128 edges, one dst
window, one src-table half):
  - gather xs rows (the ONLY per-edge DMA)
  - oh[e,d] one-hot of dst built on DVE (tensor_scalar is_equal)
  - ohT via PE transpose + PSUM->SBUF copy
  - combT[col,e] built on the PE in PSUM: emb8@oh8T + xdw@ohT + xs^T
    (kills the xd gather and the xd DRAM table of the old design)
  - lrelu via ACT Prelu(alpha=0.2) -> combT fp16 SBUF
  - logits = combT.T @ att_blk; ex = exp(logits - 4) (softmax-shift safe)
  - wgt = xs * ex (broadcast mul); scatter-accumulate per dst window via
    one-hot matmul into PSUM [sum_ex | sum_ex*xs]
  - window flush: agg/sum, @W_out (gamma-scaled), +beta +residual (all on
    PE), LayerNorm with rstd = exp(-0.5*ln(var+eps)) so the whole slab
    loop shares one ACT function table.
Output returned fp16 -> cast to fp32 on host.
"""
import numpy as np
from contextlib import ExitStack
from dataclasses import dataclass

import concourse.bass as bass
import concourse.tile as tile
from concourse import bacc, mybir
from concourse.masks import make_identity

P = 128
HID = 128
H = 4
HD = 32
NET = 8
EPS_LN = 1e-5
MAXCALL = 4096     # max idxs per dma_gather call
DEAD = -5.0        # dst_rel for padding slots
SPLIT = 25088      # src table half split (both halves < 32768 rows)
EXP_BIAS = -4.0    # constant logit shift (softmax invariant)


@dataclass
class Geo:
    N: int
    n_cores: int
    slab_w: int = 3

    @property
    def npc(self):
        return self.N // self.n_cores

    @property
    def nw(self):
        return (self.npc + P - 1) // P

    @property
    def nslab(self):
        return (self.nw + self.slab_w - 1) // self.slab_w

    @property
    def n_pad(self):   # table rows, 392*128 for N=50000
        return ((self.N + 1023) // 1024) * 1024


def wrap_idx(idx, cols):
    n = idx.shape[0]
    assert n % 16 == 0
    w = np.zeros((P, cols), dtype=np.int16)
    if n:
        t16 = idx.reshape(n // 16, 16).T
        for g in range(8):
            w[g * 16:(g + 1) * 16, :n // 16] = t16
    return w


def host_prep(g: Geo, node_embeddings, edge_index, edge_type, task_embedding,
              W_src, b_src, W_dst, b_dst, edge_emb, att,
              W_out, b_out, norm_w, norm_b, W_film, b_film):
    """Pure index work + tiny constant folding; O(N*HID)/O(E*HID) float
    math runs on device."""
    src = np.asarray(edge_index[0], dtype=np.int64)
    dst = np.asarray(edge_index[1], dtype=np.int64)
    et = np.asarray(edge_type, dtype=np.int64)
    npc, nw = g.npc, g.nw

    order = np.argsort(dst, kind="stable")
    src, dst, et = src[order], dst[order], et[order]
    core_of = dst // npc

    buckets = {}
    for c in range(g.n_cores):
        m = core_of == c
        cs, cd, ce = src[m], dst[m] - c * npc, et[m]
        for w in range(nw):
            wm = (cd // P) == w
            ws_, wd, we = cs[wm], cd[wm] - w * P, ce[wm]
            lo = ws_ < SPLIT
            buckets[(c, w, 0)] = (ws_[lo], wd[lo], we[lo])
            buckets[(c, w, 1)] = (ws_[~lo] - SPLIT, wd[~lo], we[~lo])

    caps = np.zeros((nw, 2), dtype=np.int64)
    for w in range(nw):
        for h in range(2):
            mx = max(len(buckets[(c, w, h)][0]) for c in range(g.n_cores))
            caps[w, h] = (mx + P - 1) // P
        if caps[w, 0] + caps[w, 1] == 0:
            caps[w, 0] = 1   # ensure every window gets a scatter group

    # ---- schedule ---------------------------------------------------------
    sched_slabs = []
    total_chunks = 0
    for s in range(g.nslab):
        ws = list(range(s * g.slab_w, min((s + 1) * g.slab_w, nw)))
        chunks = []            # (win_local, half, slot)
        calls = {0: [], 1: []}
        slot = 0
        for h in (0, 1):
            run = 0
            run_start = slot
            for w in ws:
                for _ in range(caps[w, h]):
                    chunks.append((w - ws[0], h, slot))
                    slot += 1
                    run += P
                    if run == MAXCALL:
                        calls[h].append((run_start, run))
                        run, run_start = 0, slot
            if run:
                calls[h].append((run_start, run))
        sched_slabs.append(dict(windows=ws, chunks=chunks, calls=calls,
                                chunk0=total_chunks))
        total_chunks += len(chunks)

    lo_cols = max(16, sum(n for sl in sched_slabs
                          for (_, n) in sl["calls"][0]) // 16)
    hi_cols = max(16, sum(n for sl in sched_slabs
                          for (_, n) in sl["calls"][1]) // 16)

    # ---- shared constants -------------------------------------------------
    # nodeT in p-major permuted column order: column (i*1024 + s*128 + p)
    # holds node (p*(n_pad//128) + i*8 + s), so that table-build block i
    # stores contiguous 2KB runs per partition and table row r == node r.
    npart_rows = g.n_pad // P          # rows per partition in p-major table
    nodeT_pm = np.zeros((HID, g.n_pad), dtype=np.float16)
    ne32 = np.asarray(node_embeddings, np.float32)
    cols = np.arange(g.n_pad)
    i_blk, rem = cols // 1024, cols % 1024
    s_sub, p_sub = rem // P, rem % P
    node_of_col = p_sub * npart_rows + i_blk * 8 + s_sub
    valid = node_of_col < g.N
    nodeT_pm[:, cols[valid]] = ne32[node_of_col[valid]].T.astype(np.float16)

    emb_eff = (np.asarray(edge_emb, np.float64)
               + np.asarray(b_src, np.float64)[None, :]
               + np.asarray(b_dst, np.float64)[None, :]).astype(np.float16)
    att_blk = np.zeros((HID, H), dtype=np.float16)
    for h in range(H):
        att_blk[h * HD:(h + 1) * HD, h] = np.asarray(att, np.float32)[h]

    consts = dict(
        nodeT_pm=nodeT_pm,
        W_src=np.asarray(W_src, np.float32).astype(np.float16),
        W_dst=np.asarray(W_dst, np.float32).astype(np.float16),
        W_out=np.asarray(W_out, np.float32).astype(np.float16),
        W_film=np.asarray(W_film, np.float32).astype(np.float16),
        b_film=np.asarray(b_film, np.float32).reshape(1, 2 * HID),
        b_out=np.asarray(b_out, np.float32).reshape(1, HID),
        task=np.asarray(task_embedding, np.float32).reshape(HID, 1)
            .astype(np.float16),
        emb8=emb_eff,                      # [8, HID] f16
        att_blk=att_blk,
    )
    skip_norm = bool(np.all(np.asarray(norm_w) == 1.0)
                     and np.all(np.asarray(norm_b) == 0.0))
    if not skip_norm:
        consts["normw"] = np.asarray(norm_w, np.float32).reshape(1, HID)
        consts["normb"] = np.asarray(norm_b, np.float32).reshape(1, HID)

    # ---- per-core arrays --------------------------------------------------
    in_maps = []
    for c in range(g.n_cores):
        lo_l, hi_l = [], []
        dstr = np.full((P, total_chunks), DEAD, dtype=np.float32)
        oh8T = np.zeros((NET, total_chunks * P), dtype=np.float16)
        ci = 0
        for sl in sched_slabs:
            ws0 = sl["windows"][0]
            per_half = {0: [], 1: []}
            nth = {}
            for (wl, h, slot) in sl["chunks"]:
                w = ws0 + wl
                es, ed, ee = buckets[(c, w, h)]
                k = nth.get((wl, h), 0)
                nth[(wl, h)] = k + 1
                sl_src = np.zeros(P, dtype=np.int64)
                n = min(P, max(0, len(es) - k * P))
                if n > 0:
                    sl_src[:n] = es[k * P:k * P + n]
                    dstr[:n, ci] = ed[k * P:k * P + n]
                    oh8T[ee[k * P:k * P + n], ci * P + np.arange(n)] = 1.0
                per_half[h].append(sl_src)
                ci += 1
            lo_l.extend(per_half[0])
            hi_l.extend(per_half[1])
        lo_i = (np.concatenate(lo_l) if lo_l else np.zeros(0, np.int64))
        hi_i = (np.concatenate(hi_l) if hi_l else np.zeros(0, np.int64))
        assert lo_i.max(initial=0) < SPLIT <= 32767 + 1
        assert hi_i.max(initial=0) < 32768

        m = dict(consts)
        no = np.zeros((nw * P, HID), dtype=np.float16)
        no[:npc] = ne32[c * npc:(c + 1) * npc].astype(np.float16)
        m["node_own"] = no                      # [nw*P, HID] f16
        m["lo_idx"] = wrap_idx(lo_i.astype(np.int16), lo_cols)
        m["hi_idx"] = wrap_idx(hi_i.astype(np.int16), hi_cols)
        m["dstr"] = dstr
        m["oh8T"] = oh8T
        in_maps.append(m)

    sched = dict(slabs=sched_slabs, caps=caps, total_chunks=total_chunks,
                 lo_cols=lo_cols, hi_cols=hi_cols, skip_norm=skip_norm)
    return sched, in_maps


def build_program(g: Geo, sched):
    nc = bacc.Bacc("TRN2", target_bir_lowering=False, debug=False,
                   num_devices=g.n_cores, num_swdge_queues=4)
    f16, f32 = mybir.dt.float16, mybir.dt.float32
    AF = mybir.ActivationFunctionType
    OP = mybir.AluOpType
    npc, nw = g.npc, g.nw
    total_chunks = sched["total_chunks"]
    lo_cols, hi_cols = sched["lo_cols"], sched["hi_cols"]
    cmax = max(len(sl["chunks"]) for sl in sched["slabs"])

    def din(name, shape, dt):
        return nc.dram_tensor(name, shape, dt, kind="ExternalInput").ap()

    nodeT_pm = din("nodeT_pm", [HID, g.n_pad], f16)
    node_own = din("node_own", [nw * P, HID], f16)
    W_src = din("W_src", [HID, HID], f16)
    W_dst = din("W_dst", [HID, HID], f16)
    W_out = din("W_out", [HID, HID], f16)
    W_film = din("W_film", [HID, 2 * HID], f16)
    b_film = din("b_film", [1, 2 * HID], f32)
    b_out = din("b_out", [1, HID], f32)
    task = din("task", [HID, 1], f16)
    emb8_d = din("emb8", [NET, HID], f16)
    att_d = din("att_blk", [HID, H], f16)
    lo_idx = din("lo_idx", [P, lo_cols], mybir.dt.int16)
    hi_idx = din("hi_idx", [P, hi_cols], mybir.dt.int16)
    dstr_d = din("dstr", [P, total_chunks], f32)
    oh8T_d = din("oh8T", [NET, total_chunks * P], f16)
    out = nc.dram_tensor("out", [nw * P, HID], f16, kind="ExternalOutput").ap()

    xsrc_pm = nc.dram_tensor("xsrc_pm", [g.n_pad, HID], f16).ap()

    with tile.TileContext(nc, trace_sim=False) as tc, ExitStack() as ctx:
        cpool = ctx.enter_context(tc.tile_pool(name="consts", bufs=1))
        bpool = ctx.enter_context(tc.tile_pool(name="build", bufs=3))
        spool = ctx.enter_context(tc.tile_pool(name="xs", bufs=3))
        hpool = ctx.enter_context(tc.tile_pool(name="oh", bufs=4))
        h8pool = ctx.enter_context(tc.tile_pool(name="oh8p", bufs=2))
        wpool = ctx.enter_context(tc.tile_pool(name="work", bufs=2))
        fpool = ctx.enter_context(tc.tile_pool(name="flush", bufs=3))
        opool = ctx.enter_context(tc.tile_pool(name="ostage", bufs=2))
        psM = ctx.enter_context(tc.tile_pool(name="psM", bufs=1, space="PSUM"))
        psC = ctx.enter_context(tc.tile_pool(name="psC", bufs=2, space="PSUM"))
        psT = ctx.enter_context(tc.tile_pool(name="psT", bufs=2, space="PSUM"))
        psE = ctx.enter_context(tc.tile_pool(name="psE", bufs=1, space="PSUM"))
        psW = ctx.enter_context(tc.tile_pool(name="psW", bufs=2, space="PSUM"))

        # ---- constants ----------------------------------------------------
        ident = cpool.tile([P, P], f16)
        make_identity(nc, ident[:])
        iota16 = cpool.tile([P, P], mybir.dt.int16)
        nc.gpsimd.iota(iota16[:], pattern=[[1, P]], base=0, channel_multiplier=0)
        iota = cpool.tile([P, P], f16)
        nc.vector.tensor_copy(iota[:], iota16[:])
        ones1 = cpool.tile([1, P], f16)
        nc.vector.memset(ones1[:], 1.0)
        eps_col = cpool.tile([P, 1], f32)
        nc.vector.memset(eps_col[:], EPS_LN)
        ebias_col = cpool.tile([P, 1], f32)
        nc.vector.memset(ebias_col[:], EXP_BIAS)

        Ws = cpool.tile([HID, HID], f16)
        nc.sync.dma_start(Ws[:], W_src[:])
        Wd = cpool.tile([HID, HID], f16)
        nc.sync.dma_start(Wd[:], W_dst[:])
        Wo = cpool.tile([HID, HID], f16)
        nc.sync.dma_start(Wo[:], W_out[:])
        Wf = cpool.tile([HID, 2 * HID], f16)
        nc.sync.dma_start(Wf[:], W_film[:])
        emb8 = cpool.tile([NET, HID], f16)
        nc.sync.dma_start(emb8[:], emb8_d[:])
        att_sb = cpool.tile([HID, H], f16)
        nc.sync.dma_start(att_sb[:], att_d[:])
        task_sb = cpool.tile([HID, 1], f16)
        nc.sync.dma_start(task_sb[:], task[:])
        bfilm_sb = cpool.tile([1, 2 * HID], f32)
        nc.sync.dma_start(bfilm_sb[:], b_film[:])
        bout_sb = cpool.tile([1, HID], f32)
        nc.sync.dma_start(bout_sb[:], b_out[:])
        lo_sb = cpool.tile([P, lo_cols], mybir.dt.int16)
        nc.sync.dma_start(lo_sb[:], lo_idx[:])
        hi_sb = cpool.tile([P, hi_cols], mybir.dt.int16)
        nc.sync.dma_start(hi_sb[:], hi_idx[:])
        dstr_sb = cpool.tile([P, total_chunks], f32)
        nc.sync.dma_start(dstr_sb[:], dstr_d[:])

        # node_own as [P, nw, HID] f16: partition p, window w -> node w*P+p
        node_own_sb = cpool.tile([P, nw, HID], f16)
        nc.sync.dma_start(node_own_sb[:],
                          node_own[:].rearrange("(w p) h -> p w h", p=P))

        # ---- FiLM (tanh table, done first) --------------------------------
        ps_f = psM.tile([1, 2 * HID], f32, space="PSUM", tag="pm")
        nc.tensor.matmul(out=ps_f[:], lhsT=task_sb[:], rhs=Wf[:],
                         start=True, stop=True)
        film = cpool.tile([1, 2 * HID], f32)
        nc.vector.tensor_add(film[:], ps_f[:], bfilm_sb[:])
        gam_t = cpool.tile([1, HID], f32)
        nc.scalar.activation(gam_t[:], film[:, :HID], AF.Tanh)
        gam16 = cpool.tile([1, HID], f16)
        nc.vector.tensor_scalar(gam16[:], gam_t[:], 0.5, 1.0, OP.mult, OP.add)
        tmpb = cpool.tile([1, HID], f32)
        nc.vector.tensor_mul(tmpb[:], bout_sb[:], gam16[:])
        beta16 = cpool.tile([1, HID], f16)
        nc.vector.tensor_add(beta16[:], tmpb[:], film[:, HID:])
        ps_g = psM.tile([P, HID], f32, space="PSUM", tag="pm")
        nc.tensor.matmul(out=ps_g[:], lhsT=ones1[:], rhs=gam16[:],
                         start=True, stop=True)
        gam_rep = cpool.tile([P, HID], f16)
        nc.vector.tensor_copy(gam_rep[:], ps_g[:])
        Wosc = cpool.tile([HID, HID], f16)
        nc.vector.tensor_mul(Wosc[:], Wo[:], gam_rep[:])

        if not sched["skip_norm"]:
            nw_dr = din("normw", [1, HID], f32)
            nb_dr = din("normb", [1, HID], f32)
            nw_sb = cpool.tile([1, HID], f32)
            nc.sync.dma_start(nw_sb[:], nw_dr[:])
            nb_sb = cpool.tile([1, HID], f32)
            nc.sync.dma_start(nb_sb[:], nb_dr[:])
            ps_w = psM.tile([P, 2 * HID], f32, space="PSUM", tag="pm")
            nc.tensor.matmul(out=ps_w[:, 0:HID], lhsT=ones1[:], rhs=nw_sb[:],
                             start=True, stop=True, skip_group_check=True)
            nc.tensor.matmul(out=ps_w[:, HID:], lhsT=ones1[:], rhs=nb_sb[:],
                             start=True, stop=True, skip_group_check=True)
            wb_rep = cpool.tile([P, 2 * HID], f32)
            nc.vector.tensor_copy(wb_rep[:], ps_w[:])

        # ---- xsrc table build (p-major, 2KB elems) ------------------------
        def _copy_act(o, i):
            nc.scalar.activation(o, i, AF.Copy)

        copy_rr = [lambda o, i: nc.vector.tensor_copy(o, i), _copy_act]
        for i in range(g.n_pad // 1024):
            nt = bpool.tile([HID, 1024], f16, tag="nt")
            nc.sync.dma_start(nt[:], nodeT_pm[:, i * 1024:(i + 1) * 1024])
            xt = bpool.tile([P, 8, HID], f16, tag="xt")
            for r in range(2):
                ps = psC.tile([P, 512], f32, space="PSUM", tag="pc")
                for j in range(4):
                    nc.tensor.matmul(out=ps[:, j * HID:(j + 1) * HID],
                                     lhsT=nt[:, (r * 4 + j) * P:(r * 4 + j + 1) * P],
                                     rhs=Ws[:], start=True, stop=True,
                                     skip_group_check=True)
                copy_rr[(2 * i + r) % 2](
                    xt[:, r * 4:(r + 1) * 4, :],
                    ps[:].rearrange("p (s h) -> p s h", s=4))
            nc.sync.dma_start(
                xsrc_pm[:].rearrange("(p j) h -> p j h", p=P)
                [:, i * 8:(i + 1) * 8, :], xt[:])
        xsrc_flat = xsrc_pm

        # ---- xdw_all: x_dst per window, kept in SBUF ----------------------
        xdw_all = cpool.tile([P, nw, HID], f16)
        for w in range(nw):
            pst = psT.tile([P, P], f16, space="PSUM", tag="pt")
            nc.tensor.transpose(out=pst[:], in_=node_own_sb[:, w, :],
                                identity=ident[:])
            noT = bpool.tile([HID, P], f16, tag="noT")
            copy_rr[w % 2](noT[:], pst[:])
            ps = psM.tile([P, HID], f32, space="PSUM", tag="pm")
            nc.tensor.matmul(out=ps[:], lhsT=noT[:], rhs=Wd[:],
                             start=True, stop=True)
            copy_rr[(w + 1) % 2](xdw_all[:, w, :], ps[:])

        # ---- edge slabs ----------------------------------------------------
        off16 = {0: 0, 1: 0}
        qn = [0]

        def nextq():
            qn[0] = (qn[0] + 1) % 4
            return qn[0]

        sl0 = sched["slabs"][0]
        C00 = len(sl0["chunks"])
        t8 = h8pool.tile([NET, C00 * P], f16, tag="oh8", name="oh8_0")
        nc.sync.dma_start(t8[:], oh8T_d[:, 0:C00 * P])
        oh8_tiles = [t8]
        pending_flush = [None]

        for s, sl in enumerate(sched["slabs"]):
            ws = sl["windows"]
            nwin = len(ws)
            chunks = sl["chunks"]
            C = len(chunks)
            c0 = sl["chunk0"]
            ngrp = (C + 3) // 4

            # per-slab type one-hots [8, C*128], prefetched one slab ahead
            oh8_sb = oh8_tiles.pop(0)
            if s + 1 < len(sched["slabs"]):
                nsl = sched["slabs"][s + 1]
                nC = len(nsl["chunks"])
                nt8 = h8pool.tile([NET, nC * P], f16, tag="oh8",
                                  name=f"oh8_{s + 1}")
                nc.sync.dma_start(
                    nt8[:], oh8T_d[:, nsl["chunk0"] * P:
                                   (nsl["chunk0"] + nC) * P])
                oh8_tiles.append(nt8)

            # gathers (xs rows; only per-edge DMA)
            xs_t = spool.tile([P, C, HID], f16, tag="xs")
            for h in (0, 1):
                base = SPLIT if h == 1 else 0
                idx_sb = lo_sb if h == 0 else hi_sb
                for (slot_off, n) in sl["calls"][h]:
                    if n == 0:
                        continue
                    nc.gpsimd.dma_gather(
                        out_ap=xs_t[:, slot_off:slot_off + n // P, :],
                        in_ap=xsrc_flat[base:g.n_pad, :],
                        idxs_ap=idx_sb[:, off16[h]:off16[h] + n // 16],
                        num_idxs=n, num_idxs_reg=n, elem_size=HID,
                        single_packet=(n <= 1024), queue_num=nextq(),
                    )
                    off16[h] += n // 16

            # one-hot builds (DVE 4x tensor_scalar)
            oh_all = hpool.tile([P, C, P], f16, tag="oh")
            for c in range(C):
                eng = nc.vector if c % 3 != 2 else nc.gpsimd
                eng.tensor_scalar(oh_all[:, c, :], iota[:],
                                  dstr_sb[:, c0 + c:c0 + c + 1], None,
                                  OP.is_equal)

            # ohT via PE transpose + grouped PSUM->SBUF copy
            ohT_all = hpool.tile([P, C, P], f16, tag="ohT")
            for gi in range(ngrp):
                gn = min(4, C - gi * 4)
                pst = psT.tile([P, 4, P], f16, space="PSUM", tag="pt")
                for k in range(gn):
                    nc.tensor.transpose(out=pst[:, k, :],
                                        in_=oh_all[:, gi * 4 + k, :],
                                        identity=ident[:])
                if gi % 2 == 0:
                    nc.vector.tensor_copy(ohT_all[:, gi * 4:gi * 4 + gn, :],
                                          pst[:, :gn, :])
                else:
                    nc.scalar.activation(ohT_all[:, gi * 4:gi * 4 + gn, :],
                                         pst[:, :gn, :], AF.Copy)

            # combT groups: emb8@oh8T + xdw@ohT + xs^T, then lrelu
            combT = wpool.tile([P, C, HID], f16, tag="combT")
            for gi in range(ngrp):
                gn = min(4, C - gi * 4)
                pc = psC.tile([P, 4, HID], f32, space="PSUM", tag="pc")
                nc.tensor.matmul(
                    out=pc[:, :gn, :].rearrange("p a b -> p (a b)"),
                    lhsT=emb8[:],
                    rhs=oh8_sb[:, (gi * 4) * P:(gi * 4 + gn) * P],
                    start=True, stop=False, skip_group_check=True)
                k = 0
                while k < gn:
                    c = gi * 4 + k
                    wl = chunks[c][0]
                    k2 = k + 1
                    while k2 < gn and chunks[gi * 4 + k2][0] == wl:
                        k2 += 1
                    nc.tensor.matmul(
                        out=pc[:, k:k2, :].rearrange("p a b -> p (a b)"),
                        lhsT=xdw_all[:, ws[0] + wl, :],
                        rhs=ohT_all[:, c:gi * 4 + k2, :]
                        .rearrange("p a b -> p (a b)"),
                        start=False, stop=False, skip_group_check=True)
                    k = k2
                for k in range(gn):
                    c = gi * 4 + k
                    nc.tensor.matmul(out=pc[:, k, :],
                                     lhsT=xs_t[:, c, :], rhs=ident[:],
                                     start=False, stop=True,
                                     skip_group_check=True)
                nc.scalar.activation(combT[:, gi * 4:gi * 4 + gn, :],
                                     pc[:, :gn, :], AF.Prelu, alpha=0.2)

            # logits + exp
            ex_ps = psE.tile([P, C, H], f32, space="PSUM", tag="pe")
            for c in range(C):
                nc.tensor.matmul(out=ex_ps[:, c, :], lhsT=combT[:, c, :],
                                 rhs=att_sb[:], start=True, stop=True,
                                 skip_group_check=True)
            rhs_t = wpool.tile([P, C, 4 + HID], f16, tag="rhs")
            nc.scalar.activation(rhs_t[:, :, 0:4], ex_ps[:], AF.Exp,
                                 bias=ebias_col[:])

            # weighted messages
            nc.vector.tensor_mul(
                rhs_t[:, :, 4:].rearrange("p c (h d) -> p c h d", h=H),
                xs_t[:].rearrange("p c (h d) -> p c h d", h=H),
                rhs_t[:, :, 0:4].unsqueeze(3).broadcast_to([P, C, H, HD]))

            # scatter-accumulate per window
            win_t = [psW.tile([P, 2, 4 + HID], f32, space="PSUM", tag="win",
                              name=f"win{s}_{i}")
                     for i in range((nwin + 1) // 2)]
            win_ps = [win_t[i // 2][:, i % 2, :] for i in range(nwin)]
            # window-by-window emission: a PSUM bank holds two windows and
            # tolerates only ONE open accumulation group at a time.
            n_per_win = [0] * nwin
            for (wl, h, slot) in chunks:
                n_per_win[wl] += 1
            seen = [0] * nwin
            flush_jobs = []
            by_win = sorted(range(C), key=lambda c: (chunks[c][0], c))
            for c in by_win:
                (wl, h, slot) = chunks[c]
                first = seen[wl] == 0
                seen[wl] += 1
                last = seen[wl] == n_per_win[wl]
                nc.tensor.matmul(out=win_ps[wl], lhsT=oh_all[:, slot, :],
                                 rhs=rhs_t[:, slot, :], start=first, stop=last,
                                 skip_group_check=True)
                if last:
                    flush_jobs.append(wl)

            def make_flush(s, ws, nwin, win_ps, flush_jobs):
              def flush():
                self_flush(s, ws, nwin, win_ps, flush_jobs)
              return flush
            pending_flush.append(make_flush(s, ws, nwin, win_ps, flush_jobs))

        # final flush
        for fj in pending_flush:
            if fj is not None:
                fj()
        if False:
            # ---- flush windows (moved into self_flush) --------------------
            ysl = opool.tile([P, nwin, HID], f16, tag="ysl", name=f"ysl{s}")
            stat = fpool.tile([P, 3, nwin], f32, tag="stat", name=f"stat{s}")
            sum_y, sum_sq, rstd_sl = stat[:, 0, :], stat[:, 1, :], stat[:, 2, :]
            for wl in flush_jobs:
                w = ws[0] + wl
                pw = win_ps[wl]
                sums = fpool.tile([P, 4], f32, tag="sums")
                nc.vector.tensor_scalar(sums[:], pw[:, 0:4], 1e-12, None,
                                        OP.max)
                rec = fpool.tile([P, 4], f32, tag="rec")
                nc.vector.reciprocal(rec[:], sums[:])
                aggn = fpool.tile([P, HID], f16, tag="aggn")
                nc.vector.tensor_mul(
                    aggn[:].rearrange("p (h d) -> p h d", h=H),
                    pw[:, 4:].rearrange("p (h d) -> p h d", h=H),
                    rec[:].unsqueeze(2).broadcast_to([P, H, HD]))
                pst = psT.tile([P, P], f16, space="PSUM", tag="pt")
                nc.tensor.transpose(out=pst[:], in_=aggn[:], identity=ident[:])
                aggT = fpool.tile([HID, P], f16, tag="aggT")
                nc.scalar.activation(aggT[:], pst[:], AF.Copy)
                po = psM.tile([P, HID], f32, space="PSUM", tag="pm")
                nc.tensor.matmul(out=po[:], lhsT=aggT[:], rhs=Wosc[:],
                                 start=True, stop=False, skip_group_check=True)
                nc.tensor.matmul(out=po[:], lhsT=ones1[:], rhs=beta16[:],
                                 start=False, stop=False, skip_group_check=True)
                nc.tensor.matmul(out=po[:], lhsT=ident[:],
                                 rhs=node_own_sb[:, w, :],
                                 start=False, stop=True, skip_group_check=True)
                nc.vector.scalar_tensor_tensor(
                    out=ysl[:, wl, :], in0=po[:], scalar=0.0, in1=po[:],
                    op0=OP.mult, op1=OP.add,
                    accum_out=sum_y[:, wl:wl + 1])
                sq = fpool.tile([P, HID], f16, tag="sq")
                nc.scalar.activation(sq[:], ysl[:, wl, :], AF.Square,
                                     accum_out=sum_sq[:, wl:wl + 1])
            # rstd = exp(-0.5*ln(var+eps)); var = sumsq/H - mu^2
            mu_sl = fpool.tile([P, nwin], f32, tag="mu", name=f"mu{s}")
            nc.vector.tensor_scalar(mu_sl[:], sum_y[:], 1.0 / HID, None,
                                    OP.mult)
            var_sl = fpool.tile([P, nwin], f32, tag="var")
            nc.vector.tensor_mul(var_sl[:], mu_sl[:], mu_sl[:])
            nc.vector.tensor_scalar(var_sl[:], var_sl[:], -1.0, None, OP.mult)
            nc.vector.scalar_tensor_tensor(
                out=var_sl[:], in0=sum_sq[:], scalar=1.0 / HID, in1=var_sl[:],
                op0=OP.mult, op1=OP.add)
            # rstd = rsqrt(var+eps) via bit-trick seed + 2 Newton iters on
            # DVE (keeps ACT on a single function table: no Ln/Sqrt).
            nc.vector.tensor_scalar(var_sl[:], var_sl[:], EPS_LN, None, OP.add)
            sd_t = fpool.tile([P, nwin], f32, tag="lnv")
            nc.vector.tensor_scalar(sd_t[:].bitcast(mybir.dt.int32),
                                    var_sl[:].bitcast(mybir.dt.int32),
                                    1, None, OP.logical_shift_right)
            nc.vector.tensor_scalar(sd_t[:].bitcast(mybir.dt.int32),
                                    sd_t[:].bitcast(mybir.dt.int32),
                                    -1, 0x5F3759DF, OP.mult, OP.add)
            half_v = fpool.tile([P, nwin], f32, tag="halfv")
            nc.vector.tensor_scalar(half_v[:], var_sl[:], -0.5, None, OP.mult)
            for _ in range(2):
                y2 = fpool.tile([P, nwin], f32, tag="y2")
                nc.vector.tensor_mul(y2[:], sd_t[:], sd_t[:])
                nc.vector.tensor_mul(y2[:], y2[:], half_v[:])
                nc.vector.tensor_scalar(y2[:], y2[:], 1.5, None, OP.add)
                nc.vector.tensor_mul(sd_t[:], sd_t[:], y2[:])
            nc.vector.tensor_copy(rstd_sl, sd_t[:])
            for wl in range(nwin):
                nc.vector.tensor_scalar(ysl[:, wl, :], ysl[:, wl, :],
                                        mu_sl[:, wl:wl + 1],
                                        rstd_sl[:, wl:wl + 1],
                                        OP.subtract, OP.mult)
            if not sched["skip_norm"]:
                for wl in range(nwin):
                    nc.vector.tensor_mul(ysl[:, wl, :], ysl[:, wl, :],
                                         wb_rep[:, 0:HID])
                    nc.vector.tensor_add(ysl[:, wl, :], ysl[:, wl, :],
                                         wb_rep[:, HID:])
            nc.sync.dma_start(
                out[ws[0] * P:(ws[0] + nwin) * P, :]
                .rearrange("(w p) h -> p w h", p=P), ysl[:])

    nc.compile()
    return nc


_CACHE = {}


def kernel(**inputs):
    N = int(np.asarray(inputs["node_embeddings"]).shape[0])
    n_cores = 8
    g = Geo(N=N, n_cores=n_cores)

    sched, in_maps = host_prep(g, **{k: np.asarray(v) for k, v in inputs.items()})

    key = (N, sched["total_chunks"], tuple(int(x) for x in sched["caps"].ravel()),
           sched["skip_norm"])
    if key not in _CACHE:
        _CACHE[key] = build_program(g, sched)
    nc = _CACHE[key]

    from concourse.bass_utils import run_bass_kernel_spmd
    res = run_bass_kernel_spmd(nc, in_maps, core_ids=list(range(n_cores)))
    out = np.concatenate([np.asarray(res.results[c]["out"][:g.npc],
                                     np.float32)
                          for c in range(n_cores)], axis=0)
    return out
